# revision 4
# baseline (speedup 1.0000x reference)
"""Causal self-attention (dense transformer block) on 8 Trainium2 NeuronCores.

Sharding: core c handles batch b = c//2 and head-group g = c%2 (8 of 16 heads).
Per core: QKV projection for its heads (fp32r matmuls), causal attention with
transposed-scores softmax (keys on partitions, unnormalized exp + fused
denominator via a ones-column appended to V), and a row-parallel output
projection producing a partial [2048, 1024] that the host sums per batch pair.

All shapes hardcoded for x[4, 2048, 1024], 16 heads, head_dim 64.
"""
import sys

sys.path.insert(0, "/opt/trn_rl_repo")

import contextlib

import numpy as np

import concourse.bass as bass
import concourse.tile as tile
from concourse import mybir
from concourse.bass_utils import run_bass_kernel_spmd
from concourse.masks import make_upper_triangular

F32 = mybir.dt.float32
F32R = mybir.dt.float32r
BF16 = mybir.dt.bfloat16
EXP = mybir.ActivationFunctionType.Exp

SEQ = 2048
DM = 1024
M = 512          # per-core qkv output dims (8 heads x 64)
HD = 64
NHC = 8          # heads per core
NJT = 16         # 128-row key tiles
NTI = 4          # 512-column query slices


def _split_multiwaits(nc, limit=1):
    """walrus in this container rejects >1 sync-wait per instruction; move
    extra waits onto same-engine nops placed directly before."""
    n = 0
    for func in nc.m.functions:
        for blk in func.blocks:
            out = []
            for inst in blk.instructions:
                si = inst.sync_info
                if si is not None and len(si.on_wait) > limit:
                    waits = list(si.on_wait)
                    for w in waits[:-limit]:
                        n += 1
                        out.append(mybir.InstNoOp(
                            name=f"I-waitsplit-{n}", engine=inst.engine,
                            bass_nofuse=True,
                            sync_info=mybir.SyncInfo(on_wait=[w], on_update=[])))
                    inst.sync_info = mybir.SyncInfo(
                        on_wait=waits[-limit:], on_update=list(si.on_update))
                out.append(inst)
            blk.instructions = out
    return n


def _build_nc(repeat=1):
    nc = bass.Bass("TRN2", target_bir_lowering=False, debug=False,
                   enable_asserts=False, num_devices=1)
    xt = nc.dram_tensor("xt", [DM, SEQ], F32, kind="ExternalInput").ap()
    wq = nc.dram_tensor("wq", [DM, M], F32, kind="ExternalInput").ap()
    wk = nc.dram_tensor("wk", [DM, M], F32, kind="ExternalInput").ap()
    wv = nc.dram_tensor("wv", [DM, M], F32, kind="ExternalInput").ap()
    wp = nc.dram_tensor("wp", [M, DM], F32, kind="ExternalInput").ap()
    out = nc.dram_tensor("out", [SEQ, DM], F32, kind="ExternalOutput").ap()

    with tile.TileContext(nc) as tc:
        for rep in range(repeat):
            with contextlib.ExitStack() as ctx:
                _body(nc, tc, ctx, xt, wq, wk, wv, wp, out, rep)
    _split_multiwaits(nc)
    return nc


def _body(nc, tc, ctx, xt, wq, wk, wv, wp, out, rep=0):
    persist = ctx.enter_context(tc.tile_pool(name=f"persist{rep}", bufs=1))
    # q_t/k_t: [dmod-of-2-heads (128), head-pair, seq]
    q_sb = persist.tile([128, 4, SEQ], F32R, tag="q")
    k_sb = persist.tile([128, 4, SEQ], F32R, tag="k")
    # v (natural) + ones column: [key-partition, key-tile, head, hd+1]
    v_sb = persist.tile([128, NJT, NHC, HD + 1], BF16, tag="v")
    # attention output, transposed: [dh-of-2-heads (128), dh-tile, seq]
    y_sb = persist.tile([128, 4, SEQ], BF16, tag="y")
    tri = persist.tile([128, 128], BF16, tag="tri")

    make_upper_triangular(nc, tri[:], val=1.0, diag=True)
    nc.vector.memset(v_sb[:, :, :, HD:HD + 1], 1.0)

    # ---------------- Phase 1: QKV projections ----------------
    with tc.tile_pool(name=f"wqkv{rep}", bufs=1) as wpool, \
         tc.tile_pool(name=f"xs{rep}", bufs=2) as xpool, \
         tc.tile_pool(name=f"ps1{rep}", bufs=6, space="PSUM") as ps1:
        w_tiles = {}
        for wname, wap in (("wq", wq), ("wk", wk), ("wv", wv)):
            for dt in range(8):
                t = wpool.tile([128, M], F32R, tag=f"{wname}{dt}")
                nc.sync.dma_start(t[:], wap[128 * dt:128 * dt + 128, :].bitcast(F32R))
                w_tiles[(wname, dt)] = t
        xt_r = xt.rearrange("(dt p) s -> p dt s", p=128)
        for ss in range(4):
            xt_t = xpool.tile([128, 8, 512], F32R, tag="x")
            nc.sync.dma_start(xt_t[:], xt_r[:, :, 512 * ss:512 * ss + 512].bitcast(F32R))
            for wname, dst in (("wq", q_sb), ("wk", k_sb)):
                for mt in range(4):
                    ps = ps1.tile([128, 512], F32, tag="ps")
                    for dt in range(8):
                        nc.tensor.matmul(
                            ps[:], w_tiles[(wname, dt)][:, 128 * mt:128 * mt + 128],
                            xt_t[:, dt, :], start=(dt == 0), stop=(dt == 7))
                    nc.scalar.copy(dst[:, mt, 512 * ss:512 * ss + 512], ps[:])
            for st in range(4):
                ps = ps1.tile([128, 512], F32, tag="ps")
                for dt in range(8):
                    nc.tensor.matmul(
                        ps[:], xt_t[:, dt, 128 * st:128 * st + 128],
                        w_tiles[("wv", dt)][:], start=(dt == 0), stop=(dt == 7))
                jt = 4 * ss + st
                nc.vector.tensor_copy(
                    v_sb[:, jt, :, 0:HD], ps[:].rearrange("p (h d) -> p h d", h=NHC))

    # ---------------- Phase 2: attention + output projection ----------------
    with tc.tile_pool(name=f"wp{rep}", bufs=1) as wppool, \
         tc.tile_pool(name=f"stg{rep}", bufs=2) as stg, \
         tc.tile_pool(name=f"psb{rep}", bufs=4) as ppool, \
         tc.tile_pool(name=f"misc{rep}", bufs=4) as mpool, \
         tc.tile_pool(name=f"osb{rep}", bufs=3) as opool, \
         tc.tile_pool(name=f"dscr{rep}", bufs=8, space="DRAM") as dpool, \
         tc.tile_pool(name=f"s_ps{rep}", bufs=2, space="PSUM") as s_pool, \
         tc.tile_pool(name=f"y_ps{rep}", bufs=2, space="PSUM") as y_pool, \
         tc.tile_pool(name=f"o_ps{rep}", bufs=2, space="PSUM") as o_pool:

        wp_sb = []
        for dt in range(4):
            st_t = stg.tile([128, DM], F32, tag="stg")
            nc.sync.dma_start(st_t[:], wp[128 * dt:128 * dt + 128, :])
            wt = wppool.tile([128, DM], BF16, tag=f"wp{dt}")
            nc.vector.tensor_copy(wt[:], st_t[:])
            wp_sb.append(wt)

        for ti in range(NTI):
            n_j = 4 * (ti + 1)
            ng = n_j // 2
            for hp in range(4):
                y_tiles = [y_pool.tile([128, 512], F32, tag="y", name=f"yps_{rep}_{ti}_{hp}_{i}")
                           for i in range(2)]
                for g in range(ng):
                    for c2 in range(2):
                        h = 2 * hp + c2
                        lo = 64 * c2
                        s_ps = s_pool.tile([128, 1024], F32, tag="s")
                        for c in range(2):
                            jt = 2 * g + c
                            nc.tensor.matmul(
                                s_ps[:, 512 * c:512 * c + 512],
                                k_sb[lo:lo + 64, hp, 128 * jt:128 * jt + 128],
                                q_sb[lo:lo + 64, hp, 512 * ti:512 * ti + 512],
                                start=True, stop=True)
                        p_t = ppool.tile([128, 1024], BF16, tag="p")
                        nc.scalar.activation(p_t[:], s_ps[:], EXP, scale=0.125)
                        for c in range(2):
                            jt = 2 * g + c
                            if jt >= 4 * ti:  # straddles the causal diagonal
                                off = 128 * (jt - 4 * ti)
                                if off:
                                    nc.vector.memset(p_t[:, 512 * c:512 * c + off], 0.0)
                                band = p_t[:, 512 * c + off:512 * c + off + 128]
                                nc.vector.tensor_mul(band, band, tri[:])
                        for c in range(2):
                            jt = 2 * g + c
                            nc.tensor.matmul(
                                y_tiles[c2][0:HD + 1, :], v_sb[:, jt, h, :],
                                p_t[:, 512 * c:512 * c + 512],
                                start=(g == 0 and c == 0), stop=(jt == n_j - 1),
                                skip_group_check=True)
                for c2 in range(2):
                    lo = 64 * c2
                    y_ps = y_tiles[c2]
                    rc = mpool.tile([1, 512], F32, tag="rc")
                    nc.vector.reciprocal(rc[:], y_ps[HD:HD + 1, :])
                    scr = dpool.tile([1, 512], F32, tag="scr")
                    nc.sync.dma_start(scr[:], rc[:])
                    bc = mpool.tile([64, 512], F32, tag="bc")
                    nc.sync.dma_start(bc[:], scr[:].to_broadcast([64, 512]))
                    nc.vector.tensor_mul(
                        y_sb[lo:lo + 64, hp, 512 * ti:512 * ti + 512],
                        y_ps[0:HD, :], bc[:])
            for st in range(4):
                s0 = 512 * ti + 128 * st
                for e in range(2):
                    op = o_pool.tile([128, 512], F32, tag="o")
                    for dt in range(4):
                        nc.tensor.matmul(
                            op[:], y_sb[:, dt, s0:s0 + 128],
                            wp_sb[dt][:, 512 * e:512 * e + 512],
                            start=(dt == 0), stop=(dt == 3))
                    ot = opool.tile([128, 512], F32, tag="ot")
                    nc.vector.tensor_copy(ot[:], op[:])
                    nc.sync.dma_start(out[s0:s0 + 128, 512 * e:512 * e + 512], ot[:])


_NC = None


def _get_nc():
    global _NC
    if _NC is None:
        _NC = _build_nc()
    return _NC


def _core_inputs(x, w_qkv, w_proj, core):
    b, g = core // 2, core % 2
    ms = slice(512 * g, 512 * g + 512)
    return {
        "xt": np.ascontiguousarray(x[b].T),
        "wq": np.ascontiguousarray(w_qkv[0:1024][ms].T),
        "wk": np.ascontiguousarray(w_qkv[1024:2048][ms].T),
        "wv": np.ascontiguousarray(w_qkv[2048:3072][ms].T),
        "wp": np.ascontiguousarray(w_proj[:, ms].T),
    }


def kernel(x, w_qkv, w_proj):
    x = np.asarray(x, dtype=np.float32)
    w_qkv = np.asarray(w_qkv, dtype=np.float32)
    w_proj = np.asarray(w_proj, dtype=np.float32)
    nc = _get_nc()
    in_maps = [_core_inputs(x, w_qkv, w_proj, c) for c in range(8)]
    res = run_bass_kernel_spmd(nc, in_maps, core_ids=list(range(8)))
    out = np.empty((4, SEQ, DM), dtype=np.float32)
    for b in range(4):
        out[b] = res.results[2 * b]["out"] + res.results[2 * b + 1]["out"]
    return out


if __name__ == "__main__":
    rng = np.random.default_rng(0)
    x = rng.standard_normal((4, SEQ, DM), dtype=np.float32)
    w_qkv = (rng.random((3 * DM, DM), dtype=np.float32) - 0.5) / 16.0
    w_proj = (rng.random((DM, DM), dtype=np.float32) - 0.5) / 16.0
    y = kernel(x, w_qkv, w_proj)
    print("ok", y.shape, float(np.abs(y).mean()))


# revision 6
# speedup vs baseline: 726.7683x; 726.7683x over previous
"""Causal self-attention (dense transformer block) on 8 Trainium2 NeuronCores.

Sharding: core c handles batch b = c//2 and head-group g = c%2 (8 of 16 heads).
Per core: QKV projection for its heads (fp32r matmuls), causal attention with
transposed-scores softmax (keys on partitions, unnormalized exp + fused
denominator via a ones-column appended to V), and a row-parallel output
projection producing a partial [2048, 1024] that the host sums per batch pair.

All shapes hardcoded for x[4, 2048, 1024], 16 heads, head_dim 64.
"""
import sys

sys.path.insert(0, "/opt/trn_rl_repo")

import contextlib

import ml_dtypes
import numpy as np

import concourse.bass as bass
import concourse.tile as tile
from concourse import mybir
from concourse.bass_utils import run_bass_kernel_spmd
from concourse.masks import make_upper_triangular

F32 = mybir.dt.float32
F32R = mybir.dt.float32r
BF16 = mybir.dt.bfloat16
EXP = mybir.ActivationFunctionType.Exp

SEQ = 2048
DM = 1024
M = 512          # per-core qkv output dims (8 heads x 64)
HD = 64
NHC = 8          # heads per core
NJT = 16         # 128-row key tiles
NTI = 4          # 512-column query slices


def _split_multiwaits(nc, limit=1):
    """walrus in this container rejects >1 sync-wait per instruction; move
    extra waits onto same-engine nops placed directly before."""
    n = 0
    for func in nc.m.functions:
        for blk in func.blocks:
            out = []
            for inst in blk.instructions:
                si = inst.sync_info
                if si is not None and len(si.on_wait) > limit:
                    waits = list(si.on_wait)
                    for w in waits[:-limit]:
                        n += 1
                        out.append(mybir.InstNoOp(
                            name=f"I-waitsplit-{n}", engine=inst.engine,
                            bass_nofuse=True,
                            sync_info=mybir.SyncInfo(on_wait=[w], on_update=[])))
                    inst.sync_info = mybir.SyncInfo(
                        on_wait=waits[-limit:], on_update=list(si.on_update))
                out.append(inst)
            blk.instructions = out
    return n


def _build_nc(repeat=1):
    nc = bass.Bass("TRN2", target_bir_lowering=False, debug=False,
                   enable_asserts=False, num_devices=1)
    xt = nc.dram_tensor("xt", [DM, SEQ], F32, kind="ExternalInput").ap()
    wq = nc.dram_tensor("wq", [DM, M], F32, kind="ExternalInput").ap()
    wk = nc.dram_tensor("wk", [DM, M], F32, kind="ExternalInput").ap()
    wv = nc.dram_tensor("wv", [DM, M], F32, kind="ExternalInput").ap()
    wp = nc.dram_tensor("wp", [M, DM], BF16, kind="ExternalInput").ap()
    out = nc.dram_tensor("out", [SEQ, DM], BF16, kind="ExternalOutput").ap()

    with tile.TileContext(nc) as tc:
        for rep in range(repeat):
            with contextlib.ExitStack() as ctx:
                _body(nc, tc, ctx, xt, wq, wk, wv, wp, out, rep)
    _split_multiwaits(nc)
    return nc


def _body(nc, tc, ctx, xt, wq, wk, wv, wp, out, rep=0):
    persist = ctx.enter_context(tc.tile_pool(name=f"persist{rep}", bufs=1))
    # q_t/k_t: [dmod-of-2-heads (128), head-pair, seq]
    q_sb = persist.tile([128, 4, SEQ], F32R, tag="q")
    k_sb = persist.tile([128, 4, SEQ], F32R, tag="k")
    # v (natural) + ones column: [key-partition, key-tile, head, hd+1]
    v_sb = persist.tile([128, NJT, NHC, HD + 1], BF16, tag="v")
    # attention output, transposed: [dh-of-2-heads (128), dh-tile, seq]
    y_sb = persist.tile([128, 4, SEQ], BF16, tag="y")
    tri = persist.tile([128, 128], BF16, tag="tri")

    make_upper_triangular(nc, tri[:], val=1.0, diag=True)
    nc.gpsimd.memset(v_sb[:, :, :, HD:HD + 1], 1.0)

    # ---------------- Phase 1: QKV projections ----------------
    with tc.tile_pool(name=f"wqkv{rep}", bufs=1) as wpool, \
         tc.tile_pool(name=f"xs{rep}", bufs=2) as xpool, \
         tc.tile_pool(name=f"ps1{rep}", bufs=6, space="PSUM") as ps1:
        w_tiles = {}
        for wname, wap in (("wq", wq), ("wk", wk), ("wv", wv)):
            for dt in range(8):
                t = wpool.tile([128, M], F32R, tag=f"{wname}{dt}")
                nc.sync.dma_start(t[:], wap[128 * dt:128 * dt + 128, :].bitcast(F32R))
                w_tiles[(wname, dt)] = t
        xt_r = xt.rearrange("(dt p) s -> p dt s", p=128)
        for ss in range(4):
            xt_t = xpool.tile([128, 8, 512], F32R, tag="x")
            nc.sync.dma_start(xt_t[:], xt_r[:, :, 512 * ss:512 * ss + 512].bitcast(F32R))
            for wname, dst in (("wq", q_sb), ("wk", k_sb)):
                for mt in range(4):
                    ps = ps1.tile([128, 512], F32, tag="ps")
                    for dt in range(8):
                        nc.tensor.matmul(
                            ps[:], w_tiles[(wname, dt)][:, 128 * mt:128 * mt + 128],
                            xt_t[:, dt, :], start=(dt == 0), stop=(dt == 7))
                    nc.vector.tensor_copy(dst[:, mt, 512 * ss:512 * ss + 512], ps[:])
            for st in range(4):
                ps = ps1.tile([128, 512], F32, tag="ps")
                for dt in range(8):
                    nc.tensor.matmul(
                        ps[:], xt_t[:, dt, 128 * st:128 * st + 128],
                        w_tiles[("wv", dt)][:], start=(dt == 0), stop=(dt == 7))
                jt = 4 * ss + st
                nc.vector.tensor_copy(
                    v_sb[:, jt, :, 0:HD], ps[:].rearrange("p (h d) -> p h d", h=NHC))

    # ---------------- Phase 2: attention + output projection ----------------
    with tc.tile_pool(name=f"wp{rep}", bufs=1) as wppool, \
         tc.tile_pool(name=f"psb{rep}", bufs=6) as ppool, \
         tc.tile_pool(name=f"misc{rep}", bufs=4) as mpool, \
         tc.tile_pool(name=f"osb{rep}", bufs=3) as opool, \
         tc.tile_pool(name=f"dscr{rep}", bufs=8, space="DRAM") as dpool, \
         tc.tile_pool(name=f"s_ps{rep}", bufs=2, space="PSUM") as s_pool, \
         tc.tile_pool(name=f"y_ps{rep}", bufs=3, space="PSUM") as y_pool, \
         tc.tile_pool(name=f"o_ps{rep}", bufs=1, space="PSUM") as o_pool:

        wp_sb = []
        for dt in range(4):
            wt = wppool.tile([128, DM], BF16, tag=f"wp{dt}")
            nc.sync.dma_start(wt[:], wp[128 * dt:128 * dt + 128, :])
            wp_sb.append(wt)

        for ti in range(NTI):
            n_j = 4 * (ti + 1)
            ng = n_j // 2
            for hp in range(4):
                y_tiles = [y_pool.tile([128, 512], F32, tag="y", name=f"yps_{rep}_{ti}_{hp}_{i}")
                           for i in range(2)]
                for g in range(ng):
                    for c2 in range(2):
                        h = 2 * hp + c2
                        lo = 64 * c2
                        s_ps = s_pool.tile([128, 1024], F32, tag="s")
                        for c in range(2):
                            jt = 2 * g + c
                            nc.tensor.matmul(
                                s_ps[:, 512 * c:512 * c + 512],
                                k_sb[lo:lo + 64, hp, 128 * jt:128 * jt + 128],
                                q_sb[lo:lo + 64, hp, 512 * ti:512 * ti + 512],
                                start=True, stop=True)
                        p_t = ppool.tile([128, 1024], BF16, tag="p")
                        nc.scalar.activation(p_t[:], s_ps[:], EXP, scale=0.125)
                        for c in range(2):
                            jt = 2 * g + c
                            if jt >= 4 * ti:  # straddles the causal diagonal
                                off = 128 * (jt - 4 * ti)
                                if off:
                                    nc.gpsimd.memset(p_t[:, 512 * c:512 * c + off], 0.0)
                                band = p_t[:, 512 * c + off:512 * c + off + 128]
                                nc.vector.tensor_mul(band, band, tri[:])
                        for c in range(2):
                            jt = 2 * g + c
                            nc.tensor.matmul(
                                y_tiles[c2][0:HD + 1, :], v_sb[:, jt, h, :],
                                p_t[:, 512 * c:512 * c + 512],
                                start=(g == 0 and c == 0), stop=(jt == n_j - 1),
                                skip_group_check=True)
                for c2 in range(2):
                    lo = 64 * c2
                    y_ps = y_tiles[c2]
                    rc = mpool.tile([1, 512], F32, tag="rc")
                    nc.vector.reciprocal(rc[:], y_ps[HD:HD + 1, :])
                    scr = dpool.tile([1, 512], F32, tag="scr")
                    nc.sync.dma_start(scr[:], rc[:])
                    bc = mpool.tile([64, 512], F32, tag="bc")
                    nc.sync.dma_start(bc[:], scr[:].to_broadcast([64, 512]))
                    nc.vector.tensor_mul(
                        y_sb[lo:lo + 64, hp, 512 * ti:512 * ti + 512],
                        y_ps[0:HD, :], bc[:])
            for st in range(4):
                s0 = 512 * ti + 128 * st
                for e in range(2):
                    op = o_pool.tile([128, 512], F32, tag="o")
                    for dt in range(4):
                        nc.tensor.matmul(
                            op[:], y_sb[:, dt, s0:s0 + 128],
                            wp_sb[dt][:, 512 * e:512 * e + 512],
                            start=(dt == 0), stop=(dt == 3))
                    ot = opool.tile([128, 512], BF16, tag="ot")
                    nc.vector.tensor_copy(ot[:], op[:])
                    nc.gpsimd.dma_start(out[s0:s0 + 128, 512 * e:512 * e + 512], ot[:])


_NC = None


def _get_nc():
    global _NC
    if _NC is None:
        _NC = _build_nc()
    return _NC


def _core_inputs(x, w_qkv, w_proj, core):
    b, g = core // 2, core % 2
    ms = slice(512 * g, 512 * g + 512)
    return {
        "xt": np.ascontiguousarray(x[b].T),
        "wq": np.ascontiguousarray(w_qkv[0:1024][ms].T),
        "wk": np.ascontiguousarray(w_qkv[1024:2048][ms].T),
        "wv": np.ascontiguousarray(w_qkv[2048:3072][ms].T),
        "wp": np.ascontiguousarray(w_proj[:, ms].T.astype(ml_dtypes.bfloat16)),
    }


def kernel(x, w_qkv, w_proj):
    x = np.asarray(x, dtype=np.float32)
    w_qkv = np.asarray(w_qkv, dtype=np.float32)
    w_proj = np.asarray(w_proj, dtype=np.float32)
    nc = _get_nc()
    in_maps = [_core_inputs(x, w_qkv, w_proj, c) for c in range(8)]
    res = run_bass_kernel_spmd(nc, in_maps, core_ids=list(range(8)))
    out = np.empty((4, SEQ, DM), dtype=np.float32)
    for b in range(4):
        out[b] = (res.results[2 * b]["out"].astype(np.float32)
                  + res.results[2 * b + 1]["out"].astype(np.float32))
    return out


if __name__ == "__main__":
    rng = np.random.default_rng(0)
    x = rng.standard_normal((4, SEQ, DM), dtype=np.float32)
    w_qkv = (rng.random((3 * DM, DM), dtype=np.float32) - 0.5) / 16.0
    w_proj = (rng.random((DM, DM), dtype=np.float32) - 0.5) / 16.0
    y = kernel(x, w_qkv, w_proj)
    print("ok", y.shape, float(np.abs(y).mean()))


# revision 8
# speedup vs baseline: 796.1746x; 1.0955x over previous
"""Causal self-attention (dense transformer block) on 8 Trainium2 NeuronCores.

Sharding: core c handles batch b = c//2 and head-group g = c%2 (8 of 16 heads).
Per core: QKV projection for its heads (fp32r matmuls), causal attention with
transposed-scores softmax (keys on partitions, unnormalized exp + fused
denominator via a ones-column appended to V), and a row-parallel output
projection producing a partial [2048, 1024] that the host sums per batch pair.

All shapes hardcoded for x[4, 2048, 1024], 16 heads, head_dim 64.
"""
import sys

sys.path.insert(0, "/opt/trn_rl_repo")

import contextlib

import ml_dtypes
import numpy as np

import concourse.bass as bass
import concourse.tile as tile
from concourse import mybir
from concourse.bass_utils import run_bass_kernel_spmd
from concourse.masks import make_upper_triangular

F32 = mybir.dt.float32
F32R = mybir.dt.float32r
BF16 = mybir.dt.bfloat16
EXP = mybir.ActivationFunctionType.Exp

SEQ = 2048
DM = 1024
M = 512          # per-core qkv output dims (8 heads x 64)
HD = 64
NHC = 8          # heads per core
NJT = 16         # 128-row key tiles
NTI = 4          # 512-column query slices


def _split_multiwaits(nc, limit=1):
    """walrus in this container rejects >1 sync-wait per instruction; move
    extra waits onto same-engine nops placed directly before."""
    n = 0
    for func in nc.m.functions:
        for blk in func.blocks:
            out = []
            for inst in blk.instructions:
                si = inst.sync_info
                if si is not None and len(si.on_wait) > limit:
                    waits = list(si.on_wait)
                    for w in waits[:-limit]:
                        n += 1
                        out.append(mybir.InstNoOp(
                            name=f"I-waitsplit-{n}", engine=inst.engine,
                            bass_nofuse=True,
                            sync_info=mybir.SyncInfo(on_wait=[w], on_update=[])))
                    inst.sync_info = mybir.SyncInfo(
                        on_wait=waits[-limit:], on_update=list(si.on_update))
                out.append(inst)
            blk.instructions = out
    return n


def _build_nc(repeat=1):
    nc = bass.Bass("TRN2", target_bir_lowering=False, debug=False,
                   enable_asserts=False, num_devices=1)
    xt = nc.dram_tensor("xt", [DM, SEQ], F32, kind="ExternalInput").ap()
    wq = nc.dram_tensor("wq", [DM, M], F32, kind="ExternalInput").ap()
    wk = nc.dram_tensor("wk", [DM, M], F32, kind="ExternalInput").ap()
    wv = nc.dram_tensor("wv", [DM, M], F32, kind="ExternalInput").ap()
    wp = nc.dram_tensor("wp", [M, DM], BF16, kind="ExternalInput").ap()
    out = nc.dram_tensor("out", [SEQ, DM], BF16, kind="ExternalOutput").ap()

    with tile.TileContext(nc) as tc:
        for rep in range(repeat):
            with contextlib.ExitStack() as ctx:
                _body(nc, tc, ctx, xt, wq, wk, wv, wp, out, rep)
    _split_multiwaits(nc)
    return nc


def _body(nc, tc, ctx, xt, wq, wk, wv, wp, out, rep=0):
    persist = ctx.enter_context(tc.tile_pool(name=f"persist{rep}", bufs=1))
    # q_t/k_t: [dmod-of-2-heads (128), head-pair, seq]
    q_sb = persist.tile([128, 4, SEQ], F32R, tag="q")
    k_sb = persist.tile([128, 4, SEQ], F32R, tag="k")
    # v (natural) + ones column: [key-partition, key-tile, head, hd+1]
    v_sb = persist.tile([128, NJT, NHC, HD + 1], BF16, tag="v")
    # attention output, transposed: [dh-of-2-heads (128), dh-tile, seq]
    y_sb = persist.tile([128, 4, SEQ], BF16, tag="y")
    tri = persist.tile([128, 128], BF16, tag="tri")

    make_upper_triangular(nc, tri[:], val=1.0, diag=True)
    nc.gpsimd.memset(v_sb[:, :, :, HD:HD + 1], 1.0)

    # ---------------- Phase 1: QKV projections ----------------
    with tc.tile_pool(name=f"wqkv{rep}", bufs=1) as wpool, \
         tc.tile_pool(name=f"xs{rep}", bufs=2) as xpool, \
         tc.tile_pool(name=f"ps1{rep}", bufs=6, space="PSUM") as ps1:
        w_tiles = {}
        for wname, wap in (("wq", wq), ("wk", wk), ("wv", wv)):
            for dt in range(8):
                t = wpool.tile([128, M], F32R, tag=f"{wname}{dt}")
                nc.gpsimd.dma_start(t[:], wap[128 * dt:128 * dt + 128, :].bitcast(F32R))
                w_tiles[(wname, dt)] = t
        xt_r = xt.rearrange("(dt p) s -> p dt s", p=128)
        for ss in range(4):
            xt_t = xpool.tile([128, 8, 512], F32R, tag="x")
            for dh in range(4):
                nc.sync.dma_start(
                    xt_t[:, 2 * dh:2 * dh + 2, :],
                    xt_r[:, 2 * dh:2 * dh + 2, 512 * ss:512 * ss + 512].bitcast(F32R))
            for wname, dst in (("wq", q_sb), ("wk", k_sb)):
                for mt in range(4):
                    ps = ps1.tile([128, 512], F32, tag="ps")
                    for dt in range(8):
                        nc.tensor.matmul(
                            ps[:], w_tiles[(wname, dt)][:, 128 * mt:128 * mt + 128],
                            xt_t[:, dt, :], start=(dt == 0), stop=(dt == 7))
                    nc.vector.tensor_copy(dst[:, mt, 512 * ss:512 * ss + 512], ps[:])
            for st in range(4):
                ps = ps1.tile([128, 512], F32, tag="ps")
                for dt in range(8):
                    nc.tensor.matmul(
                        ps[:], xt_t[:, dt, 128 * st:128 * st + 128],
                        w_tiles[("wv", dt)][:], start=(dt == 0), stop=(dt == 7))
                jt = 4 * ss + st
                nc.vector.tensor_copy(
                    v_sb[:, jt, :, 0:HD], ps[:].rearrange("p (h d) -> p h d", h=NHC))

    # ---------------- Phase 2: attention + output projection ----------------
    with tc.tile_pool(name=f"wp{rep}", bufs=1) as wppool, \
         tc.tile_pool(name=f"psb{rep}", bufs=8) as ppool, \
         tc.tile_pool(name=f"misc{rep}", bufs=4) as mpool, \
         tc.tile_pool(name=f"osb{rep}", bufs=3) as opool, \
         tc.tile_pool(name=f"dscr{rep}", bufs=8, space="DRAM") as dpool, \
         tc.tile_pool(name=f"s_ps{rep}", bufs=2, space="PSUM") as s_pool, \
         tc.tile_pool(name=f"y_ps{rep}", bufs=3, space="PSUM") as y_pool, \
         tc.tile_pool(name=f"o_ps{rep}", bufs=1, space="PSUM") as o_pool:

        wp_sb = []
        for dt in range(4):
            wt = wppool.tile([128, DM], BF16, tag=f"wp{dt}")
            nc.sync.dma_start(wt[:], wp[128 * dt:128 * dt + 128, :])
            wp_sb.append(wt)

        for ti in range(NTI):
            n_j = 4 * (ti + 1)
            ng = n_j // 2
            for hp in range(4):
                y_tiles = [y_pool.tile([128, 512], F32, tag="y", name=f"yps_{rep}_{ti}_{hp}_{i}")
                           for i in range(2)]
                for g in range(ng):
                    for c2 in range(2):
                        h = 2 * hp + c2
                        lo = 64 * c2
                        s_ps = s_pool.tile([128, 1024], F32, tag="s")
                        for c in range(2):
                            jt = 2 * g + c
                            nc.tensor.matmul(
                                s_ps[:, 512 * c:512 * c + 512],
                                k_sb[lo:lo + 64, hp, 128 * jt:128 * jt + 128],
                                q_sb[lo:lo + 64, hp, 512 * ti:512 * ti + 512],
                                start=True, stop=True)
                        p_t = ppool.tile([128, 1024], BF16, tag="p")
                        nc.scalar.activation(p_t[:], s_ps[:], EXP, scale=0.125)
                        for c in range(2):
                            jt = 2 * g + c
                            if jt >= 4 * ti:  # straddles the causal diagonal
                                off = 128 * (jt - 4 * ti)
                                if off:
                                    nc.gpsimd.memset(p_t[:, 512 * c:512 * c + off], 0.0)
                                band = p_t[:, 512 * c + off:512 * c + off + 128]
                                nc.vector.tensor_mul(band, band, tri[:])
                        for c in range(2):
                            jt = 2 * g + c
                            nc.tensor.matmul(
                                y_tiles[c2][0:HD + 1, :], v_sb[:, jt, h, :],
                                p_t[:, 512 * c:512 * c + 512],
                                start=(g == 0 and c == 0), stop=(jt == n_j - 1),
                                skip_group_check=True)
                for c2 in range(2):
                    lo = 64 * c2
                    y_ps = y_tiles[c2]
                    rc = mpool.tile([1, 512], F32, tag="rc")
                    nc.vector.reciprocal(rc[:], y_ps[HD:HD + 1, :])
                    scr = dpool.tile([1, 512], F32, tag="scr")
                    nc.sync.dma_start(scr[:], rc[:])
                    bc = mpool.tile([64, 512], F32, tag="bc")
                    nc.sync.dma_start(bc[:], scr[:].to_broadcast([64, 512]))
                    nc.vector.tensor_mul(
                        y_sb[lo:lo + 64, hp, 512 * ti:512 * ti + 512],
                        y_ps[0:HD, :], bc[:])
            for st in range(4):
                s0 = 512 * ti + 128 * st
                for e in range(2):
                    op = o_pool.tile([128, 512], F32, tag="o")
                    for dt in range(4):
                        nc.tensor.matmul(
                            op[:], y_sb[:, dt, s0:s0 + 128],
                            wp_sb[dt][:, 512 * e:512 * e + 512],
                            start=(dt == 0), stop=(dt == 3))
                    ot = opool.tile([128, 512], BF16, tag="ot")
                    nc.vector.tensor_copy(ot[:], op[:])
                    nc.gpsimd.dma_start(out[s0:s0 + 128, 512 * e:512 * e + 512], ot[:])


_NC = None


def _get_nc():
    global _NC
    if _NC is None:
        _NC = _build_nc()
    return _NC


def _core_inputs(x, w_qkv, w_proj, core):
    b, g = core // 2, core % 2
    ms = slice(512 * g, 512 * g + 512)
    return {
        "xt": np.ascontiguousarray(x[b].T),
        "wq": np.ascontiguousarray(w_qkv[0:1024][ms].T),
        "wk": np.ascontiguousarray(w_qkv[1024:2048][ms].T),
        "wv": np.ascontiguousarray(w_qkv[2048:3072][ms].T),
        "wp": np.ascontiguousarray(w_proj[:, ms].T.astype(ml_dtypes.bfloat16)),
    }


def kernel(x, w_qkv, w_proj):
    x = np.asarray(x, dtype=np.float32)
    w_qkv = np.asarray(w_qkv, dtype=np.float32)
    w_proj = np.asarray(w_proj, dtype=np.float32)
    nc = _get_nc()
    in_maps = [_core_inputs(x, w_qkv, w_proj, c) for c in range(8)]
    res = run_bass_kernel_spmd(nc, in_maps, core_ids=list(range(8)))
    out = np.empty((4, SEQ, DM), dtype=np.float32)
    for b in range(4):
        out[b] = (res.results[2 * b]["out"].astype(np.float32)
                  + res.results[2 * b + 1]["out"].astype(np.float32))
    return out


if __name__ == "__main__":
    rng = np.random.default_rng(0)
    x = rng.standard_normal((4, SEQ, DM), dtype=np.float32)
    w_qkv = (rng.random((3 * DM, DM), dtype=np.float32) - 0.5) / 16.0
    w_proj = (rng.random((DM, DM), dtype=np.float32) - 0.5) / 16.0
    y = kernel(x, w_qkv, w_proj)
    print("ok", y.shape, float(np.abs(y).mean()))


# revision 14
# speedup vs baseline: 826.3652x; 1.0379x over previous
"""Causal self-attention (dense transformer block) on 8 Trainium2 NeuronCores.

Sharding: core c handles batch b = c//2 and head-group g = c%2 (8 of 16 heads).
Per core: QKV projection for its heads (fp32r matmuls), causal attention with
transposed-scores softmax (keys on partitions, unnormalized exp + fused
denominator via a ones-column appended to V), and a row-parallel output
projection producing a partial [2048, 1024] that the host sums per batch pair.

All shapes hardcoded for x[4, 2048, 1024], 16 heads, head_dim 64.
"""
import sys

sys.path.insert(0, "/opt/trn_rl_repo")

import contextlib

import ml_dtypes
import numpy as np

import concourse.bass as bass
import concourse.tile as tile
from concourse import mybir
from concourse.bass_utils import run_bass_kernel_spmd
from concourse.masks import make_upper_triangular

F32 = mybir.dt.float32
F32R = mybir.dt.float32r
BF16 = mybir.dt.bfloat16
EXP = mybir.ActivationFunctionType.Exp

SEQ = 2048
DM = 1024
M = 512          # per-core qkv output dims (8 heads x 64)
HD = 64
NHC = 8          # heads per core
NJT = 16         # 128-row key tiles
NTI = 4          # 512-column query slices


def _split_multiwaits(nc, limit=1):
    """walrus in this container rejects >1 sync-wait per instruction; move
    extra waits onto same-engine nops placed directly before."""
    n = 0
    for func in nc.m.functions:
        for blk in func.blocks:
            out = []
            for inst in blk.instructions:
                si = inst.sync_info
                if si is not None and len(si.on_wait) > limit:
                    waits = list(si.on_wait)
                    for w in waits[:-limit]:
                        n += 1
                        out.append(mybir.InstNoOp(
                            name=f"I-waitsplit-{n}", engine=inst.engine,
                            bass_nofuse=True,
                            sync_info=mybir.SyncInfo(on_wait=[w], on_update=[])))
                    inst.sync_info = mybir.SyncInfo(
                        on_wait=waits[-limit:], on_update=list(si.on_update))
                out.append(inst)
            blk.instructions = out
    return n


def _build_nc(repeat=1):
    nc = bass.Bass("TRN2", target_bir_lowering=False, debug=False,
                   enable_asserts=False, num_devices=1)
    xt = nc.dram_tensor("xt", [DM, SEQ], F32, kind="ExternalInput").ap()
    wq = nc.dram_tensor("wq", [DM, M], F32, kind="ExternalInput").ap()
    wk = nc.dram_tensor("wk", [DM, M], F32, kind="ExternalInput").ap()
    wv = nc.dram_tensor("wv", [DM, M], F32, kind="ExternalInput").ap()
    wp = nc.dram_tensor("wp", [M, DM], BF16, kind="ExternalInput").ap()
    out = nc.dram_tensor("out", [SEQ, DM], BF16, kind="ExternalOutput").ap()

    with tile.TileContext(nc) as tc:
        for rep in range(repeat):
            with contextlib.ExitStack() as ctx:
                _body(nc, tc, ctx, xt, wq, wk, wv, wp, out, rep)
    _split_multiwaits(nc)
    return nc


def _body(nc, tc, ctx, xt, wq, wk, wv, wp, out, rep=0):
    persist = ctx.enter_context(tc.tile_pool(name=f"persist{rep}", bufs=1))
    # q_t/k_t: [dmod-of-2-heads (128), head-pair, seq]
    q_sb = persist.tile([128, 4, SEQ], F32R, tag="q")
    k_sb = persist.tile([128, 4, SEQ], F32R, tag="k")
    # v (natural) + ones column: [key-partition, key-tile, head, hd+1]
    v_sb = persist.tile([128, NJT, NHC, HD + 1], BF16, tag="v")
    # attention output, transposed: [dh-of-2-heads (128), dh-tile, seq]
    y_sb = persist.tile([128, 4, SEQ], BF16, tag="y")
    tri = persist.tile([128, 128], BF16, tag="tri")

    make_upper_triangular(nc, tri[:], val=1.0, diag=True)
    nc.gpsimd.memset(v_sb[:, :, :, HD:HD + 1], 1.0)

    # ---------------- Phase 1: QKV projections ----------------
    with tc.tile_pool(name=f"wqkv{rep}", bufs=1) as wpool, \
         tc.tile_pool(name=f"xs{rep}", bufs=2) as xpool, \
         tc.tile_pool(name=f"ps1{rep}", bufs=6, space="PSUM") as ps1:
        w_tiles = {}
        for wname, wap in (("wq", wq), ("wk", wk), ("wv", wv)):
            for dt in range(8):
                t = wpool.tile([128, M], F32R, tag=f"{wname}{dt}")
                nc.gpsimd.dma_start(t[:], wap[128 * dt:128 * dt + 128, :].bitcast(F32R))
                w_tiles[(wname, dt)] = t
        xt_r = xt.rearrange("(dt p) s -> p dt s", p=128)
        for ss in range(4):
            xt_t = xpool.tile([128, 8, 512], F32R, tag="x")
            for dh in range(4):
                nc.sync.dma_start(
                    xt_t[:, 2 * dh:2 * dh + 2, :],
                    xt_r[:, 2 * dh:2 * dh + 2, 512 * ss:512 * ss + 512].bitcast(F32R))
            for wname, dst in (("wq", q_sb), ("wk", k_sb)):
                for mt in range(4):
                    ps = ps1.tile([128, 512], F32, tag="ps")
                    for dt in range(8):
                        nc.tensor.matmul(
                            ps[:], w_tiles[(wname, dt)][:, 128 * mt:128 * mt + 128],
                            xt_t[:, dt, :], start=(dt == 0), stop=(dt == 7))
                    nc.vector.tensor_copy(dst[:, mt, 512 * ss:512 * ss + 512], ps[:])
            for st in range(4):
                ps = ps1.tile([128, 512], F32, tag="ps")
                for dt in range(8):
                    nc.tensor.matmul(
                        ps[:], xt_t[:, dt, 128 * st:128 * st + 128],
                        w_tiles[("wv", dt)][:], start=(dt == 0), stop=(dt == 7))
                jt = 4 * ss + st
                nc.vector.tensor_copy(
                    v_sb[:, jt, :, 0:HD], ps[:].rearrange("p (h d) -> p h d", h=NHC))

    # ---------------- Phase 2: attention + output projection ----------------
    with tc.tile_pool(name=f"wp{rep}", bufs=1) as wppool, \
         tc.tile_pool(name=f"psb{rep}", bufs=8) as ppool, \
         tc.tile_pool(name=f"misc{rep}", bufs=4) as mpool, \
         tc.tile_pool(name=f"osb{rep}", bufs=3) as opool, \
         tc.tile_pool(name=f"dscr{rep}", bufs=8, space="DRAM") as dpool, \
         tc.tile_pool(name=f"s_ps{rep}", bufs=2, space="PSUM") as s_pool, \
         tc.tile_pool(name=f"y_ps{rep}", bufs=3, space="PSUM") as y_pool, \
         tc.tile_pool(name=f"o_ps{rep}", bufs=1, space="PSUM") as o_pool:

        wp_sb = []
        for dt in range(4):
            wt = wppool.tile([128, DM], BF16, tag=f"wp{dt}")
            nc.sync.dma_start(wt[:], wp[128 * dt:128 * dt + 128, :])
            wp_sb.append(wt)

        for ti in range(NTI):
            n_j = 4 * (ti + 1)
            ng = n_j // 2
            for hp in range(4):
                y_tiles = [y_pool.tile([128, 512], F32, tag="y", name=f"yps_{rep}_{ti}_{hp}_{i}")
                           for i in range(2)]
                for g in range(ng):
                    for c2 in range(2):
                        h = 2 * hp + c2
                        lo = 64 * c2
                        s_ps = s_pool.tile([128, 1024], F32, tag="s")
                        for c in range(2):
                            jt = 2 * g + c
                            nc.tensor.matmul(
                                s_ps[:, 512 * c:512 * c + 512],
                                k_sb[lo:lo + 64, hp, 128 * jt:128 * jt + 128],
                                q_sb[lo:lo + 64, hp, 512 * ti:512 * ti + 512],
                                start=True, stop=True)
                        p_t = ppool.tile([128, 1024], BF16, tag="p")
                        nc.scalar.activation(p_t[:], s_ps[:], EXP, scale=0.125)
                        for c in range(2):
                            jt = 2 * g + c
                            if jt >= 4 * ti:  # straddles the causal diagonal
                                off = 128 * (jt - 4 * ti)
                                band = p_t[:, 512 * c + off:512 * c + off + 128]
                                nc.vector.tensor_mul(band, band, tri[:])
                        for c in range(2):
                            jt = 2 * g + c
                            # columns below a diagonal tile's start are fully
                            # masked: skip them instead of zeroing p there
                            off = 128 * (jt - 4 * ti) if jt >= 4 * ti else 0
                            nc.tensor.matmul(
                                y_tiles[c2][0:HD + 1, off:512],
                                v_sb[:, jt, h, :],
                                p_t[:, 512 * c + off:512 * c + 512],
                                start=(g == 0 and c == 0), stop=(jt == n_j - 1),
                                skip_group_check=True)
                for c2 in range(2):
                    lo = 64 * c2
                    y_ps = y_tiles[c2]
                    rc = mpool.tile([1, 512], F32, tag="rc")
                    nc.vector.reciprocal(rc[:], y_ps[HD:HD + 1, :])
                    scr = dpool.tile([1, 512], F32, tag="scr")
                    nc.sync.dma_start(scr[:], rc[:])
                    bc = mpool.tile([64, 512], F32, tag="bc")
                    nc.sync.dma_start(bc[:], scr[:].to_broadcast([64, 512]))
                    nc.vector.tensor_mul(
                        y_sb[lo:lo + 64, hp, 512 * ti:512 * ti + 512],
                        y_ps[0:HD, :], bc[:])
            for st in range(4):
                s0 = 512 * ti + 128 * st
                for e in range(2):
                    op = o_pool.tile([128, 512], F32, tag="o")
                    for dt in range(4):
                        nc.tensor.matmul(
                            op[:], y_sb[:, dt, s0:s0 + 128],
                            wp_sb[dt][:, 512 * e:512 * e + 512],
                            start=(dt == 0), stop=(dt == 3))
                    ot = opool.tile([128, 512], BF16, tag="ot")
                    nc.vector.tensor_copy(ot[:], op[:])
                    nc.gpsimd.dma_start(out[s0:s0 + 128, 512 * e:512 * e + 512], ot[:])


_NC = None


def _get_nc():
    global _NC
    if _NC is None:
        _NC = _build_nc()
    return _NC


def _core_inputs(x, w_qkv, w_proj, core):
    b, g = core // 2, core % 2
    ms = slice(512 * g, 512 * g + 512)
    return {
        "xt": np.ascontiguousarray(x[b].T),
        "wq": np.ascontiguousarray(w_qkv[0:1024][ms].T),
        "wk": np.ascontiguousarray(w_qkv[1024:2048][ms].T),
        "wv": np.ascontiguousarray(w_qkv[2048:3072][ms].T),
        "wp": np.ascontiguousarray(w_proj[:, ms].T.astype(ml_dtypes.bfloat16)),
    }


def kernel(x, w_qkv, w_proj):
    x = np.asarray(x, dtype=np.float32)
    w_qkv = np.asarray(w_qkv, dtype=np.float32)
    w_proj = np.asarray(w_proj, dtype=np.float32)
    nc = _get_nc()
    in_maps = [_core_inputs(x, w_qkv, w_proj, c) for c in range(8)]
    res = run_bass_kernel_spmd(nc, in_maps, core_ids=list(range(8)))
    out = np.empty((4, SEQ, DM), dtype=np.float32)
    for b in range(4):
        out[b] = (res.results[2 * b]["out"].astype(np.float32)
                  + res.results[2 * b + 1]["out"].astype(np.float32))
    return out


if __name__ == "__main__":
    rng = np.random.default_rng(0)
    x = rng.standard_normal((4, SEQ, DM), dtype=np.float32)
    w_qkv = (rng.random((3 * DM, DM), dtype=np.float32) - 0.5) / 16.0
    w_proj = (rng.random((DM, DM), dtype=np.float32) - 0.5) / 16.0
    y = kernel(x, w_qkv, w_proj)
    print("ok", y.shape, float(np.abs(y).mean()))


# revision 15
# speedup vs baseline: 830.5478x; 1.0051x over previous
"""Causal self-attention (dense transformer block) on 8 Trainium2 NeuronCores.

Sharding: core c handles batch b = c//2 and head-group g = c%2 (8 of 16 heads).
Per core: QKV projection for its heads (fp32r matmuls), causal attention with
transposed-scores softmax (keys on partitions, unnormalized exp + fused
denominator via a ones-column appended to V), and a row-parallel output
projection producing a partial [2048, 1024] that the host sums per batch pair.

All shapes hardcoded for x[4, 2048, 1024], 16 heads, head_dim 64.
"""
import sys

sys.path.insert(0, "/opt/trn_rl_repo")

import contextlib

import ml_dtypes
import numpy as np

import concourse.bass as bass
import concourse.tile as tile
from concourse import mybir
from concourse.bass_utils import run_bass_kernel_spmd
from concourse.masks import make_upper_triangular

F32 = mybir.dt.float32
F32R = mybir.dt.float32r
BF16 = mybir.dt.bfloat16
EXP = mybir.ActivationFunctionType.Exp

SEQ = 2048
DM = 1024
M = 512          # per-core qkv output dims (8 heads x 64)
HD = 64
NHC = 8          # heads per core
NJT = 16         # 128-row key tiles
NTI = 4          # 512-column query slices


def _split_multiwaits(nc, limit=1):
    """walrus in this container rejects >1 sync-wait per instruction; move
    extra waits onto same-engine nops placed directly before."""
    n = 0
    for func in nc.m.functions:
        for blk in func.blocks:
            out = []
            for inst in blk.instructions:
                si = inst.sync_info
                if si is not None and len(si.on_wait) > limit:
                    waits = list(si.on_wait)
                    for w in waits[:-limit]:
                        n += 1
                        out.append(mybir.InstNoOp(
                            name=f"I-waitsplit-{n}", engine=inst.engine,
                            bass_nofuse=True,
                            sync_info=mybir.SyncInfo(on_wait=[w], on_update=[])))
                    inst.sync_info = mybir.SyncInfo(
                        on_wait=waits[-limit:], on_update=list(si.on_update))
                out.append(inst)
            blk.instructions = out
    return n


def _build_nc(repeat=1):
    nc = bass.Bass("TRN2", target_bir_lowering=False, debug=False,
                   enable_asserts=False, num_devices=1)
    xt = nc.dram_tensor("xt", [DM, SEQ], F32, kind="ExternalInput").ap()
    wq = nc.dram_tensor("wq", [DM, M], F32, kind="ExternalInput").ap()
    wk = nc.dram_tensor("wk", [DM, M], F32, kind="ExternalInput").ap()
    wv = nc.dram_tensor("wv", [DM, M], F32, kind="ExternalInput").ap()
    wp = nc.dram_tensor("wp", [M, DM], BF16, kind="ExternalInput").ap()
    out = nc.dram_tensor("out", [SEQ, DM], BF16, kind="ExternalOutput").ap()

    with tile.TileContext(nc) as tc:
        for rep in range(repeat):
            with contextlib.ExitStack() as ctx:
                _body(nc, tc, ctx, xt, wq, wk, wv, wp, out, rep)
    _split_multiwaits(nc)
    return nc


def _body(nc, tc, ctx, xt, wq, wk, wv, wp, out, rep=0):
    persist = ctx.enter_context(tc.tile_pool(name=f"persist{rep}", bufs=1))
    # q_t/k_t: [dmod-of-2-heads (128), head-pair, seq]
    q_sb = persist.tile([128, 4, SEQ], F32R, tag="q")
    k_sb = persist.tile([128, 4, SEQ], F32R, tag="k")
    # v (natural) + ones column: [key-partition, key-tile, head, hd+1]
    v_sb = persist.tile([128, NJT, NHC, HD + 1], BF16, tag="v")
    tri = persist.tile([128, 128], BF16, tag="tri")

    make_upper_triangular(nc, tri[:], val=1.0, diag=True)
    nc.gpsimd.memset(v_sb[:, :, :, HD:HD + 1], 1.0)

    # Score banks and p tiles coexist with the QKV-phase pools (PSUM split
    # 4+4) so QK+exp for early query slices overlap the tail of the QKV
    # phase; only the PV/proj accumulators wait for the QKV banks to free.
    s_pool = ctx.enter_context(
        tc.tile_pool(name=f"s_ps{rep}", bufs=2, space="PSUM"))
    ppool = ctx.enter_context(tc.tile_pool(name=f"psb{rep}", bufs=12))

    # ---------------- Phase 1: QKV projections ----------------
    with tc.tile_pool(name=f"wqkv{rep}", bufs=1) as wpool, \
         tc.tile_pool(name=f"xs{rep}", bufs=2) as xpool, \
         tc.tile_pool(name=f"ps1{rep}", bufs=4, space="PSUM") as ps1:
        w_tiles = {}
        for wname, wap in (("wq", wq), ("wk", wk), ("wv", wv)):
            for dt in range(8):
                t = wpool.tile([128, M], F32R, tag=f"{wname}{dt}")
                nc.gpsimd.dma_start(t[:], wap[128 * dt:128 * dt + 128, :].bitcast(F32R))
                w_tiles[(wname, dt)] = t
        xt_r = xt.rearrange("(dt p) s -> p dt s", p=128)
        for ss in range(4):
            xt_t = xpool.tile([128, 8, 512], F32R, tag="x")
            for dh in range(4):
                nc.sync.dma_start(
                    xt_t[:, 2 * dh:2 * dh + 2, :],
                    xt_r[:, 2 * dh:2 * dh + 2, 512 * ss:512 * ss + 512].bitcast(F32R))
            for wname, dst in (("wq", q_sb), ("wk", k_sb)):
                for mt in range(4):
                    ps = ps1.tile([128, 512], F32, tag="ps")
                    for dt in range(8):
                        nc.tensor.matmul(
                            ps[:], w_tiles[(wname, dt)][:, 128 * mt:128 * mt + 128],
                            xt_t[:, dt, :], start=(dt == 0), stop=(dt == 7))
                    nc.vector.tensor_copy(dst[:, mt, 512 * ss:512 * ss + 512], ps[:])
            for st in range(4):
                ps = ps1.tile([128, 512], F32, tag="ps")
                for dt in range(8):
                    nc.tensor.matmul(
                        ps[:], xt_t[:, dt, 128 * st:128 * st + 128],
                        w_tiles[("wv", dt)][:], start=(dt == 0), stop=(dt == 7))
                jt = 4 * ss + st
                nc.vector.tensor_copy(
                    v_sb[:, jt, :, 0:HD], ps[:].rearrange("p (h d) -> p h d", h=NHC))

    # ---------------- Phase 2: attention + output projection ----------------
    with tc.tile_pool(name=f"wp{rep}", bufs=1) as wppool, \
         tc.tile_pool(name=f"p2sb{rep}", bufs=1) as p2pool, \
         tc.tile_pool(name=f"misc{rep}", bufs=4) as mpool, \
         tc.tile_pool(name=f"osb{rep}", bufs=3) as opool, \
         tc.tile_pool(name=f"dscr{rep}", bufs=8, space="DRAM") as dpool, \
         tc.tile_pool(name=f"y_ps{rep}", bufs=3, space="PSUM") as y_pool, \
         tc.tile_pool(name=f"o_ps{rep}", bufs=1, space="PSUM") as o_pool:

        # attention output, transposed: [dh-of-2-heads (128), dh-tile, seq]
        y_sb = p2pool.tile([128, 4, SEQ], BF16, tag="y")
        wp_sb = []
        for dt in range(4):
            wt = wppool.tile([128, DM], BF16, tag=f"wp{dt}")
            nc.sync.dma_start(wt[:], wp[128 * dt:128 * dt + 128, :])
            wp_sb.append(wt)

        for ti in range(NTI):
            n_j = 4 * (ti + 1)
            ng = n_j // 2
            for hp in range(4):
                y_tiles = [y_pool.tile([128, 512], F32, tag="y", name=f"yps_{rep}_{ti}_{hp}_{i}")
                           for i in range(2)]
                for g in range(ng):
                    for c2 in range(2):
                        h = 2 * hp + c2
                        lo = 64 * c2
                        s_ps = s_pool.tile([128, 1024], F32, tag="s")
                        for c in range(2):
                            jt = 2 * g + c
                            nc.tensor.matmul(
                                s_ps[:, 512 * c:512 * c + 512],
                                k_sb[lo:lo + 64, hp, 128 * jt:128 * jt + 128],
                                q_sb[lo:lo + 64, hp, 512 * ti:512 * ti + 512],
                                start=True, stop=True)
                        p_t = ppool.tile([128, 1024], BF16, tag="p")
                        nc.scalar.activation(p_t[:], s_ps[:], EXP, scale=0.125)
                        for c in range(2):
                            jt = 2 * g + c
                            if jt >= 4 * ti:  # straddles the causal diagonal
                                off = 128 * (jt - 4 * ti)
                                band = p_t[:, 512 * c + off:512 * c + off + 128]
                                nc.vector.tensor_mul(band, band, tri[:])
                        for c in range(2):
                            jt = 2 * g + c
                            # columns below a diagonal tile's start are fully
                            # masked: skip them instead of zeroing p there
                            off = 128 * (jt - 4 * ti) if jt >= 4 * ti else 0
                            nc.tensor.matmul(
                                y_tiles[c2][0:HD + 1, off:512],
                                v_sb[:, jt, h, :],
                                p_t[:, 512 * c + off:512 * c + 512],
                                start=(g == 0 and c == 0), stop=(jt == n_j - 1),
                                skip_group_check=True)
                for c2 in range(2):
                    lo = 64 * c2
                    y_ps = y_tiles[c2]
                    rc = mpool.tile([1, 512], F32, tag="rc")
                    nc.vector.reciprocal(rc[:], y_ps[HD:HD + 1, :])
                    scr = dpool.tile([1, 512], F32, tag="scr")
                    nc.sync.dma_start(scr[:], rc[:])
                    bc = mpool.tile([64, 512], F32, tag="bc")
                    nc.sync.dma_start(bc[:], scr[:].to_broadcast([64, 512]))
                    nc.vector.tensor_mul(
                        y_sb[lo:lo + 64, hp, 512 * ti:512 * ti + 512],
                        y_ps[0:HD, :], bc[:])
            for st in range(4):
                s0 = 512 * ti + 128 * st
                for e in range(2):
                    op = o_pool.tile([128, 512], F32, tag="o")
                    for dt in range(4):
                        nc.tensor.matmul(
                            op[:], y_sb[:, dt, s0:s0 + 128],
                            wp_sb[dt][:, 512 * e:512 * e + 512],
                            start=(dt == 0), stop=(dt == 3))
                    ot = opool.tile([128, 512], BF16, tag="ot")
                    nc.vector.tensor_copy(ot[:], op[:])
                    nc.gpsimd.dma_start(out[s0:s0 + 128, 512 * e:512 * e + 512], ot[:])


_NC = None


def _get_nc():
    global _NC
    if _NC is None:
        _NC = _build_nc()
    return _NC


def _core_inputs(x, w_qkv, w_proj, core):
    b, g = core // 2, core % 2
    ms = slice(512 * g, 512 * g + 512)
    return {
        "xt": np.ascontiguousarray(x[b].T),
        "wq": np.ascontiguousarray(w_qkv[0:1024][ms].T),
        "wk": np.ascontiguousarray(w_qkv[1024:2048][ms].T),
        "wv": np.ascontiguousarray(w_qkv[2048:3072][ms].T),
        "wp": np.ascontiguousarray(w_proj[:, ms].T.astype(ml_dtypes.bfloat16)),
    }


def kernel(x, w_qkv, w_proj):
    x = np.asarray(x, dtype=np.float32)
    w_qkv = np.asarray(w_qkv, dtype=np.float32)
    w_proj = np.asarray(w_proj, dtype=np.float32)
    nc = _get_nc()
    in_maps = [_core_inputs(x, w_qkv, w_proj, c) for c in range(8)]
    res = run_bass_kernel_spmd(nc, in_maps, core_ids=list(range(8)))
    out = np.empty((4, SEQ, DM), dtype=np.float32)
    for b in range(4):
        out[b] = (res.results[2 * b]["out"].astype(np.float32)
                  + res.results[2 * b + 1]["out"].astype(np.float32))
    return out


if __name__ == "__main__":
    rng = np.random.default_rng(0)
    x = rng.standard_normal((4, SEQ, DM), dtype=np.float32)
    w_qkv = (rng.random((3 * DM, DM), dtype=np.float32) - 0.5) / 16.0
    w_proj = (rng.random((DM, DM), dtype=np.float32) - 0.5) / 16.0
    y = kernel(x, w_qkv, w_proj)
    print("ok", y.shape, float(np.abs(y).mean()))


# revision 19
# speedup vs baseline: 833.2306x; 1.0032x over previous
"""Causal self-attention (dense transformer block) on 8 Trainium2 NeuronCores.

Sharding: core c handles batch b = c//2 and head-group g = c%2 (8 of 16 heads).
Per core: QKV projection for its heads (fp32r matmuls), causal attention with
transposed-scores softmax (keys on partitions, unnormalized exp + fused
denominator via a ones-column appended to V), and a row-parallel output
projection producing a partial [2048, 1024] that the host sums per batch pair.

All shapes hardcoded for x[4, 2048, 1024], 16 heads, head_dim 64.
"""
import sys

sys.path.insert(0, "/opt/trn_rl_repo")

import contextlib

import ml_dtypes
import numpy as np

import concourse.bass as bass
import concourse.tile as tile
from concourse import mybir
from concourse.bass_utils import run_bass_kernel_spmd
from concourse.masks import make_upper_triangular

F32 = mybir.dt.float32
F32R = mybir.dt.float32r
BF16 = mybir.dt.bfloat16
EXP = mybir.ActivationFunctionType.Exp

SEQ = 2048
DM = 1024
M = 512          # per-core qkv output dims (8 heads x 64)
HD = 64
NHC = 8          # heads per core
NJT = 16         # 128-row key tiles
NTI = 4          # 512-column query slices


def _split_multiwaits(nc, limit=1):
    """walrus in this container rejects >1 sync-wait per instruction; move
    extra waits onto same-engine nops placed directly before."""
    n = 0
    for func in nc.m.functions:
        for blk in func.blocks:
            out = []
            for inst in blk.instructions:
                si = inst.sync_info
                if si is not None and len(si.on_wait) > limit:
                    waits = list(si.on_wait)
                    for w in waits[:-limit]:
                        n += 1
                        out.append(mybir.InstNoOp(
                            name=f"I-waitsplit-{n}", engine=inst.engine,
                            bass_nofuse=True,
                            sync_info=mybir.SyncInfo(on_wait=[w], on_update=[])))
                    inst.sync_info = mybir.SyncInfo(
                        on_wait=waits[-limit:], on_update=list(si.on_update))
                out.append(inst)
            blk.instructions = out
    return n


def _build_nc(repeat=1):
    nc = bass.Bass("TRN2", target_bir_lowering=False, debug=False,
                   enable_asserts=False, num_devices=1)
    xt = nc.dram_tensor("xt", [DM, SEQ], F32, kind="ExternalInput").ap()
    wq = nc.dram_tensor("wq", [DM, M], F32, kind="ExternalInput").ap()
    wk = nc.dram_tensor("wk", [DM, M], F32, kind="ExternalInput").ap()
    wv = nc.dram_tensor("wv", [DM, M], F32, kind="ExternalInput").ap()
    wp = nc.dram_tensor("wp", [M, DM], BF16, kind="ExternalInput").ap()
    out = nc.dram_tensor("out", [SEQ, DM], BF16, kind="ExternalOutput").ap()

    with tile.TileContext(nc) as tc:
        for rep in range(repeat):
            with contextlib.ExitStack() as ctx:
                _body(nc, tc, ctx, xt, wq, wk, wv, wp, out, rep)
    _split_multiwaits(nc)
    return nc


def _body(nc, tc, ctx, xt, wq, wk, wv, wp, out, rep=0):
    persist = ctx.enter_context(tc.tile_pool(name=f"persist{rep}", bufs=1))
    # q_t/k_t: [dmod-of-2-heads (128), head-pair, seq]
    q_sb = persist.tile([128, 4, SEQ], F32R, tag="q")
    k_sb = persist.tile([128, 4, SEQ], F32R, tag="k")
    # v (natural) + ones column: [key-partition, key-tile, head, hd+1]
    v_sb = persist.tile([128, NJT, NHC, HD + 1], BF16, tag="v")
    tri = persist.tile([128, 128], BF16, tag="tri")

    make_upper_triangular(nc, tri[:], val=1.0, diag=True)
    nc.gpsimd.memset(v_sb[:, :, :, HD:HD + 1], 1.0)

    # Score banks and p tiles coexist with the QKV-phase pools (PSUM split
    # 4+4) so QK+exp for early query slices overlap the tail of the QKV
    # phase; only the PV/proj accumulators wait for the QKV banks to free.
    s_pool = ctx.enter_context(
        tc.tile_pool(name=f"s_ps{rep}", bufs=2, space="PSUM"))
    ppool = ctx.enter_context(tc.tile_pool(name=f"psb{rep}", bufs=12))

    # ---------------- Phase 1: QKV projections ----------------
    with tc.tile_pool(name=f"wqkv{rep}", bufs=1) as wpool, \
         tc.tile_pool(name=f"xs{rep}", bufs=2) as xpool, \
         tc.tile_pool(name=f"ps1{rep}", bufs=4, space="PSUM") as ps1:
        w_tiles = {}
        for wname, wap in (("wq", wq), ("wk", wk), ("wv", wv)):
            for dt in range(8):
                t = wpool.tile([128, M], F32R, tag=f"{wname}{dt}")
                nc.gpsimd.dma_start(t[:], wap[128 * dt:128 * dt + 128, :].bitcast(F32R))
                w_tiles[(wname, dt)] = t
        xt_r = xt.rearrange("(dt p) s -> p dt s", p=128)
        for ss in range(4):
            xt_t = xpool.tile([128, 8, 512], F32R, tag="x")
            for dh in range(4):
                nc.sync.dma_start(
                    xt_t[:, 2 * dh:2 * dh + 2, :],
                    xt_r[:, 2 * dh:2 * dh + 2, 512 * ss:512 * ss + 512].bitcast(F32R))
            for wname, dst in (("wq", q_sb), ("wk", k_sb)):
                for mt in range(4):
                    ps = ps1.tile([128, 512], F32, tag="ps")
                    for dt in range(8):
                        nc.tensor.matmul(
                            ps[:], w_tiles[(wname, dt)][:, 128 * mt:128 * mt + 128],
                            xt_t[:, dt, :], start=(dt == 0), stop=(dt == 7))
                    nc.vector.tensor_copy(dst[:, mt, 512 * ss:512 * ss + 512], ps[:])
            for st in range(4):
                ps = ps1.tile([128, 512], F32, tag="ps")
                for dt in range(8):
                    nc.tensor.matmul(
                        ps[:], xt_t[:, dt, 128 * st:128 * st + 128],
                        w_tiles[("wv", dt)][:], start=(dt == 0), stop=(dt == 7))
                jt = 4 * ss + st
                nc.vector.tensor_copy(
                    v_sb[:, jt, :, 0:HD], ps[:].rearrange("p (h d) -> p h d", h=NHC))

    # ---------------- Phase 2: attention + output projection ----------------
    with tc.tile_pool(name=f"wp{rep}", bufs=1) as wppool, \
         tc.tile_pool(name=f"p2sb{rep}", bufs=1) as p2pool, \
         tc.tile_pool(name=f"misc{rep}", bufs=4) as mpool, \
         tc.tile_pool(name=f"osb{rep}", bufs=3) as opool, \
         tc.tile_pool(name=f"dscr{rep}", bufs=8, space="DRAM") as dpool, \
         tc.tile_pool(name=f"y_ps{rep}", bufs=3, space="PSUM") as y_pool, \
         tc.tile_pool(name=f"o_ps{rep}", bufs=1, space="PSUM") as o_pool:

        # attention output, transposed: [dh-of-2-heads (128), dh-tile, seq]
        y_sb = p2pool.tile([128, 4, SEQ], BF16, tag="y")
        wp_sb = []
        for dt in range(4):
            wt = wppool.tile([128, DM], BF16, tag=f"wp{dt}")
            nc.sync.dma_start(wt[:], wp[128 * dt:128 * dt + 128, :])
            wp_sb.append(wt)

        for ti in range(NTI):
            n_j = 4 * (ti + 1)
            ng = n_j // 2
            for hp in range(4):
                y_tiles = [y_pool.tile([128, 512], F32, tag="y", name=f"yps_{rep}_{ti}_{hp}_{i}")
                           for i in range(2)]
                for g in range(ng):
                    for c2 in range(2):
                        h = 2 * hp + c2
                        lo = 64 * c2
                        s_ps = s_pool.tile([128, 1024], F32, tag="s")
                        for c in range(2):
                            jt = 2 * g + c
                            nc.tensor.matmul(
                                s_ps[:, 512 * c:512 * c + 512],
                                k_sb[lo:lo + 64, hp, 128 * jt:128 * jt + 128],
                                q_sb[lo:lo + 64, hp, 512 * ti:512 * ti + 512],
                                start=True, stop=True)
                        p_t = ppool.tile([128, 1024], BF16, tag="p")
                        # columns before the first tile's causal start are
                        # never read by the narrowed PV: skip them in exp too
                        off0 = 128 * (2 * g - 4 * ti) if 2 * g >= 4 * ti else 0
                        nc.scalar.activation(p_t[:, off0:1024],
                                             s_ps[:, off0:1024],
                                             EXP, scale=0.125)
                        for c in range(2):
                            jt = 2 * g + c
                            if jt >= 4 * ti:  # straddles the causal diagonal
                                off = 128 * (jt - 4 * ti)
                                band = p_t[:, 512 * c + off:512 * c + off + 128]
                                nc.vector.tensor_mul(band, band, tri[:])
                        for c in range(2):
                            jt = 2 * g + c
                            # columns below a diagonal tile's start are fully
                            # masked: skip them instead of zeroing p there
                            off = 128 * (jt - 4 * ti) if jt >= 4 * ti else 0
                            nc.tensor.matmul(
                                y_tiles[c2][0:HD + 1, off:512],
                                v_sb[:, jt, h, :],
                                p_t[:, 512 * c + off:512 * c + 512],
                                start=(g == 0 and c == 0), stop=(jt == n_j - 1),
                                skip_group_check=True)
                for c2 in range(2):
                    lo = 64 * c2
                    y_ps = y_tiles[c2]
                    rc = mpool.tile([1, 512], F32, tag="rc")
                    nc.vector.reciprocal(rc[:], y_ps[HD:HD + 1, :])
                    scr = dpool.tile([1, 512], F32, tag="scr")
                    nc.sync.dma_start(scr[:], rc[:])
                    bc = mpool.tile([64, 512], F32, tag="bc")
                    nc.sync.dma_start(bc[:], scr[:].to_broadcast([64, 512]))
                    nc.vector.tensor_mul(
                        y_sb[lo:lo + 64, hp, 512 * ti:512 * ti + 512],
                        y_ps[0:HD, :], bc[:])
            for st in range(4):
                s0 = 512 * ti + 128 * st
                for e in range(2):
                    op = o_pool.tile([128, 512], F32, tag="o")
                    for dt in range(4):
                        nc.tensor.matmul(
                            op[:], y_sb[:, dt, s0:s0 + 128],
                            wp_sb[dt][:, 512 * e:512 * e + 512],
                            start=(dt == 0), stop=(dt == 3))
                    ot = opool.tile([128, 512], BF16, tag="ot")
                    nc.vector.tensor_copy(ot[:], op[:])
                    nc.gpsimd.dma_start(out[s0:s0 + 128, 512 * e:512 * e + 512], ot[:])


_NC = None


def _get_nc():
    global _NC
    if _NC is None:
        _NC = _build_nc()
    return _NC


def _core_inputs(x, w_qkv, w_proj, core):
    b, g = core // 2, core % 2
    ms = slice(512 * g, 512 * g + 512)
    return {
        "xt": np.ascontiguousarray(x[b].T),
        "wq": np.ascontiguousarray(w_qkv[0:1024][ms].T),
        "wk": np.ascontiguousarray(w_qkv[1024:2048][ms].T),
        "wv": np.ascontiguousarray(w_qkv[2048:3072][ms].T),
        "wp": np.ascontiguousarray(w_proj[:, ms].T.astype(ml_dtypes.bfloat16)),
    }


def kernel(x, w_qkv, w_proj):
    x = np.asarray(x, dtype=np.float32)
    w_qkv = np.asarray(w_qkv, dtype=np.float32)
    w_proj = np.asarray(w_proj, dtype=np.float32)
    nc = _get_nc()
    in_maps = [_core_inputs(x, w_qkv, w_proj, c) for c in range(8)]
    res = run_bass_kernel_spmd(nc, in_maps, core_ids=list(range(8)))
    out = np.empty((4, SEQ, DM), dtype=np.float32)
    for b in range(4):
        out[b] = (res.results[2 * b]["out"].astype(np.float32)
                  + res.results[2 * b + 1]["out"].astype(np.float32))
    return out


if __name__ == "__main__":
    rng = np.random.default_rng(0)
    x = rng.standard_normal((4, SEQ, DM), dtype=np.float32)
    w_qkv = (rng.random((3 * DM, DM), dtype=np.float32) - 0.5) / 16.0
    w_proj = (rng.random((DM, DM), dtype=np.float32) - 0.5) / 16.0
    y = kernel(x, w_qkv, w_proj)
    print("ok", y.shape, float(np.abs(y).mean()))


# revision 20
# speedup vs baseline: 833.9125x; 1.0008x over previous
"""Causal self-attention (dense transformer block) on 8 Trainium2 NeuronCores.

Sharding: core c handles batch b = c//2 and head-group g = c%2 (8 of 16 heads).
Per core: QKV projection for its heads (fp32r matmuls), causal attention with
transposed-scores softmax (keys on partitions, unnormalized exp + fused
denominator via a ones-column appended to V), and a row-parallel output
projection producing a partial [2048, 1024] that the host sums per batch pair.

All shapes hardcoded for x[4, 2048, 1024], 16 heads, head_dim 64.
"""
import sys

sys.path.insert(0, "/opt/trn_rl_repo")

import contextlib

import ml_dtypes
import numpy as np

import concourse.bass as bass
import concourse.tile as tile
from concourse import mybir
from concourse.bass_utils import run_bass_kernel_spmd
from concourse.masks import make_upper_triangular

F32 = mybir.dt.float32
F32R = mybir.dt.float32r
BF16 = mybir.dt.bfloat16
EXP = mybir.ActivationFunctionType.Exp

SEQ = 2048
DM = 1024
M = 512          # per-core qkv output dims (8 heads x 64)
HD = 64
NHC = 8          # heads per core
NJT = 16         # 128-row key tiles
NTI = 4          # 512-column query slices


def _split_multiwaits(nc, limit=1):
    """walrus in this container rejects >1 sync-wait per instruction; move
    extra waits onto same-engine nops placed directly before."""
    n = 0
    for func in nc.m.functions:
        for blk in func.blocks:
            out = []
            for inst in blk.instructions:
                si = inst.sync_info
                if si is not None and len(si.on_wait) > limit:
                    waits = list(si.on_wait)
                    for w in waits[:-limit]:
                        n += 1
                        out.append(mybir.InstNoOp(
                            name=f"I-waitsplit-{n}", engine=inst.engine,
                            bass_nofuse=True,
                            sync_info=mybir.SyncInfo(on_wait=[w], on_update=[])))
                    inst.sync_info = mybir.SyncInfo(
                        on_wait=waits[-limit:], on_update=list(si.on_update))
                out.append(inst)
            blk.instructions = out
    return n


def _build_nc(repeat=1):
    nc = bass.Bass("TRN2", target_bir_lowering=False, debug=False,
                   enable_asserts=False, num_devices=1)
    xt = nc.dram_tensor("xt", [DM, SEQ], F32, kind="ExternalInput").ap()
    wq = nc.dram_tensor("wq", [DM, M], F32, kind="ExternalInput").ap()
    wk = nc.dram_tensor("wk", [DM, M], F32, kind="ExternalInput").ap()
    wv = nc.dram_tensor("wv", [DM, M], F32, kind="ExternalInput").ap()
    wp = nc.dram_tensor("wp", [M, DM], BF16, kind="ExternalInput").ap()
    out = nc.dram_tensor("out", [SEQ, DM], BF16, kind="ExternalOutput").ap()

    with tile.TileContext(nc) as tc:
        for rep in range(repeat):
            with contextlib.ExitStack() as ctx:
                _body(nc, tc, ctx, xt, wq, wk, wv, wp, out, rep)
    _split_multiwaits(nc)
    return nc


def _body(nc, tc, ctx, xt, wq, wk, wv, wp, out, rep=0):
    persist = ctx.enter_context(tc.tile_pool(name=f"persist{rep}", bufs=1))
    # q_t/k_t: [dmod-of-2-heads (128), head-pair, seq]
    q_sb = persist.tile([128, 4, SEQ], F32R, tag="q")
    k_sb = persist.tile([128, 4, SEQ], F32R, tag="k")
    # v (natural) + ones column: [key-partition, key-tile, head, hd+1]
    v_sb = persist.tile([128, NJT, NHC, HD + 1], BF16, tag="v")
    tri = persist.tile([128, 128], BF16, tag="tri")

    make_upper_triangular(nc, tri[:], val=1.0, diag=True)
    nc.gpsimd.memset(v_sb[:, :, :, HD:HD + 1], 1.0)

    # Score banks and p tiles coexist with the QKV-phase pools (PSUM split
    # 4+4) so QK+exp for early query slices overlap the tail of the QKV
    # phase; only the PV/proj accumulators wait for the QKV banks to free.
    s_pool = ctx.enter_context(
        tc.tile_pool(name=f"s_ps{rep}", bufs=2, space="PSUM"))
    ppool = ctx.enter_context(tc.tile_pool(name=f"psb{rep}", bufs=14))

    # ---------------- Phase 1: QKV projections ----------------
    with tc.tile_pool(name=f"wqkv{rep}", bufs=1) as wpool, \
         tc.tile_pool(name=f"xs{rep}", bufs=2) as xpool, \
         tc.tile_pool(name=f"ps1{rep}", bufs=4, space="PSUM") as ps1:
        w_tiles = {}
        for wname, wap in (("wq", wq), ("wk", wk), ("wv", wv)):
            for dt in range(8):
                t = wpool.tile([128, M], F32R, tag=f"{wname}{dt}")
                nc.gpsimd.dma_start(t[:], wap[128 * dt:128 * dt + 128, :].bitcast(F32R))
                w_tiles[(wname, dt)] = t
        xt_r = xt.rearrange("(dt p) s -> p dt s", p=128)
        for ss in range(4):
            xt_t = xpool.tile([128, 8, 512], F32R, tag="x")
            for dh in range(4):
                nc.sync.dma_start(
                    xt_t[:, 2 * dh:2 * dh + 2, :],
                    xt_r[:, 2 * dh:2 * dh + 2, 512 * ss:512 * ss + 512].bitcast(F32R))
            for wname, dst in (("wq", q_sb), ("wk", k_sb)):
                for mt in range(4):
                    ps = ps1.tile([128, 512], F32, tag="ps")
                    for dt in range(8):
                        nc.tensor.matmul(
                            ps[:], w_tiles[(wname, dt)][:, 128 * mt:128 * mt + 128],
                            xt_t[:, dt, :], start=(dt == 0), stop=(dt == 7))
                    nc.vector.tensor_copy(dst[:, mt, 512 * ss:512 * ss + 512], ps[:])
            for st in range(4):
                ps = ps1.tile([128, 512], F32, tag="ps")
                for dt in range(8):
                    nc.tensor.matmul(
                        ps[:], xt_t[:, dt, 128 * st:128 * st + 128],
                        w_tiles[("wv", dt)][:], start=(dt == 0), stop=(dt == 7))
                jt = 4 * ss + st
                nc.vector.tensor_copy(
                    v_sb[:, jt, :, 0:HD], ps[:].rearrange("p (h d) -> p h d", h=NHC))

    # ---------------- Phase 2: attention + output projection ----------------
    with tc.tile_pool(name=f"wp{rep}", bufs=1) as wppool, \
         tc.tile_pool(name=f"p2sb{rep}", bufs=1) as p2pool, \
         tc.tile_pool(name=f"misc{rep}", bufs=4) as mpool, \
         tc.tile_pool(name=f"osb{rep}", bufs=3) as opool, \
         tc.tile_pool(name=f"dscr{rep}", bufs=8, space="DRAM") as dpool, \
         tc.tile_pool(name=f"y_ps{rep}", bufs=3, space="PSUM") as y_pool, \
         tc.tile_pool(name=f"o_ps{rep}", bufs=1, space="PSUM") as o_pool:

        # attention output, transposed: [dh-of-2-heads (128), dh-tile, seq]
        y_sb = p2pool.tile([128, 4, SEQ], BF16, tag="y")
        wp_sb = []
        for dt in range(4):
            wt = wppool.tile([128, DM], BF16, tag=f"wp{dt}")
            nc.sync.dma_start(wt[:], wp[128 * dt:128 * dt + 128, :])
            wp_sb.append(wt)

        for ti in range(NTI):
            n_j = 4 * (ti + 1)
            ng = n_j // 2
            for hp in range(4):
                y_tiles = [y_pool.tile([128, 512], F32, tag="y", name=f"yps_{rep}_{ti}_{hp}_{i}")
                           for i in range(2)]
                for g in range(ng):
                    for c2 in range(2):
                        h = 2 * hp + c2
                        lo = 64 * c2
                        s_ps = s_pool.tile([128, 1024], F32, tag="s")
                        for c in range(2):
                            jt = 2 * g + c
                            nc.tensor.matmul(
                                s_ps[:, 512 * c:512 * c + 512],
                                k_sb[lo:lo + 64, hp, 128 * jt:128 * jt + 128],
                                q_sb[lo:lo + 64, hp, 512 * ti:512 * ti + 512],
                                start=True, stop=True)
                        p_t = ppool.tile([128, 1024], BF16, tag="p")
                        # columns before the first tile's causal start are
                        # never read by the narrowed PV: skip them in exp too
                        off0 = 128 * (2 * g - 4 * ti) if 2 * g >= 4 * ti else 0
                        nc.scalar.activation(p_t[:, off0:1024],
                                             s_ps[:, off0:1024],
                                             EXP, scale=0.125)
                        for c in range(2):
                            jt = 2 * g + c
                            if jt >= 4 * ti:  # straddles the causal diagonal
                                off = 128 * (jt - 4 * ti)
                                band = p_t[:, 512 * c + off:512 * c + off + 128]
                                nc.vector.tensor_mul(band, band, tri[:])
                        for c in range(2):
                            jt = 2 * g + c
                            # columns below a diagonal tile's start are fully
                            # masked: skip them instead of zeroing p there
                            off = 128 * (jt - 4 * ti) if jt >= 4 * ti else 0
                            nc.tensor.matmul(
                                y_tiles[c2][0:HD + 1, off:512],
                                v_sb[:, jt, h, :],
                                p_t[:, 512 * c + off:512 * c + 512],
                                start=(g == 0 and c == 0), stop=(jt == n_j - 1),
                                skip_group_check=True)
                for c2 in range(2):
                    lo = 64 * c2
                    y_ps = y_tiles[c2]
                    rc = mpool.tile([1, 512], F32, tag="rc")
                    nc.vector.reciprocal(rc[:], y_ps[HD:HD + 1, :])
                    scr = dpool.tile([1, 512], F32, tag="scr")
                    nc.sync.dma_start(scr[:], rc[:])
                    bc = mpool.tile([64, 512], F32, tag="bc")
                    nc.sync.dma_start(bc[:], scr[:].to_broadcast([64, 512]))
                    nc.vector.tensor_mul(
                        y_sb[lo:lo + 64, hp, 512 * ti:512 * ti + 512],
                        y_ps[0:HD, :], bc[:])
            for st in range(4):
                s0 = 512 * ti + 128 * st
                for e in range(2):
                    op = o_pool.tile([128, 512], F32, tag="o")
                    for dt in range(4):
                        nc.tensor.matmul(
                            op[:], y_sb[:, dt, s0:s0 + 128],
                            wp_sb[dt][:, 512 * e:512 * e + 512],
                            start=(dt == 0), stop=(dt == 3))
                    ot = opool.tile([128, 512], BF16, tag="ot")
                    nc.vector.tensor_copy(ot[:], op[:])
                    nc.gpsimd.dma_start(out[s0:s0 + 128, 512 * e:512 * e + 512], ot[:])


_NC = None


def _get_nc():
    global _NC
    if _NC is None:
        _NC = _build_nc()
    return _NC


def _core_inputs(x, w_qkv, w_proj, core):
    b, g = core // 2, core % 2
    ms = slice(512 * g, 512 * g + 512)
    return {
        "xt": np.ascontiguousarray(x[b].T),
        "wq": np.ascontiguousarray(w_qkv[0:1024][ms].T),
        "wk": np.ascontiguousarray(w_qkv[1024:2048][ms].T),
        "wv": np.ascontiguousarray(w_qkv[2048:3072][ms].T),
        "wp": np.ascontiguousarray(w_proj[:, ms].T.astype(ml_dtypes.bfloat16)),
    }


def kernel(x, w_qkv, w_proj):
    x = np.asarray(x, dtype=np.float32)
    w_qkv = np.asarray(w_qkv, dtype=np.float32)
    w_proj = np.asarray(w_proj, dtype=np.float32)
    nc = _get_nc()
    in_maps = [_core_inputs(x, w_qkv, w_proj, c) for c in range(8)]
    res = run_bass_kernel_spmd(nc, in_maps, core_ids=list(range(8)))
    out = np.empty((4, SEQ, DM), dtype=np.float32)
    for b in range(4):
        out[b] = (res.results[2 * b]["out"].astype(np.float32)
                  + res.results[2 * b + 1]["out"].astype(np.float32))
    return out


if __name__ == "__main__":
    rng = np.random.default_rng(0)
    x = rng.standard_normal((4, SEQ, DM), dtype=np.float32)
    w_qkv = (rng.random((3 * DM, DM), dtype=np.float32) - 0.5) / 16.0
    w_proj = (rng.random((DM, DM), dtype=np.float32) - 0.5) / 16.0
    y = kernel(x, w_qkv, w_proj)
    print("ok", y.shape, float(np.abs(y).mean()))


# revision 26
# speedup vs baseline: 907.7753x; 1.0886x over previous
"""Causal self-attention (dense transformer block) on 8 Trainium2 NeuronCores.

Sharding: core c handles batch b = c//2 and head-group g = c%2 (8 of 16 heads).
Per core: QKV projection for its heads (fp32r matmuls), causal attention with
transposed-scores softmax (keys on partitions, unnormalized exp + fused
denominator via a ones-column appended to V), and a row-parallel output
projection producing a partial [2048, 1024] that the host sums per batch pair.

All shapes hardcoded for x[4, 2048, 1024], 16 heads, head_dim 64.
"""
import sys

sys.path.insert(0, "/opt/trn_rl_repo")

import contextlib

import ml_dtypes
import numpy as np

import concourse.bass as bass
import concourse.tile as tile
from concourse import mybir
from concourse.bass_utils import run_bass_kernel_spmd
from concourse.masks import make_upper_triangular

F32 = mybir.dt.float32
F32R = mybir.dt.float32r
BF16 = mybir.dt.bfloat16
EXP = mybir.ActivationFunctionType.Exp

SEQ = 2048
DM = 1024
M = 512          # per-core qkv output dims (8 heads x 64)
HD = 64
NHC = 8          # heads per core
NJT = 16         # 128-row key tiles
NTI = 4          # 512-column query slices


def _split_multiwaits(nc, limit=1):
    """walrus in this container rejects >1 sync-wait per instruction; move
    extra waits onto same-engine nops placed directly before."""
    n = 0
    for func in nc.m.functions:
        for blk in func.blocks:
            out = []
            for inst in blk.instructions:
                si = inst.sync_info
                if si is not None and len(si.on_wait) > limit:
                    waits = list(si.on_wait)
                    for w in waits[:-limit]:
                        n += 1
                        out.append(mybir.InstNoOp(
                            name=f"I-waitsplit-{n}", engine=inst.engine,
                            bass_nofuse=True,
                            sync_info=mybir.SyncInfo(on_wait=[w], on_update=[])))
                    inst.sync_info = mybir.SyncInfo(
                        on_wait=waits[-limit:], on_update=list(si.on_update))
                out.append(inst)
            blk.instructions = out
    return n


def _build_nc(repeat=1):
    nc = bass.Bass("TRN2", target_bir_lowering=False, debug=False,
                   enable_asserts=False, num_devices=1)
    xt = nc.dram_tensor("xt", [DM, SEQ], BF16, kind="ExternalInput").ap()
    wq = nc.dram_tensor("wq", [DM, M], BF16, kind="ExternalInput").ap()
    wk = nc.dram_tensor("wk", [DM, M], BF16, kind="ExternalInput").ap()
    wv = nc.dram_tensor("wv", [DM, M], BF16, kind="ExternalInput").ap()
    wp = nc.dram_tensor("wp", [M, DM], BF16, kind="ExternalInput").ap()
    out = nc.dram_tensor("out", [SEQ, DM], BF16, kind="ExternalOutput").ap()

    with tile.TileContext(nc) as tc:
        for rep in range(repeat):
            with contextlib.ExitStack() as ctx:
                _body(nc, tc, ctx, xt, wq, wk, wv, wp, out, rep)
    _split_multiwaits(nc)
    return nc


def _body(nc, tc, ctx, xt, wq, wk, wv, wp, out, rep=0):
    persist = ctx.enter_context(tc.tile_pool(name=f"persist{rep}", bufs=1))
    # q_t/k_t: [dmod-of-2-heads (128), head-pair, seq]
    q_sb = persist.tile([128, 4, SEQ], BF16, tag="q")
    k_sb = persist.tile([128, 4, SEQ], BF16, tag="k")
    # v (natural) + ones column: [key-partition, key-tile, head, hd+1]
    v_sb = persist.tile([128, NJT, NHC, HD + 1], BF16, tag="v")
    tri = persist.tile([128, 128], BF16, tag="tri")

    make_upper_triangular(nc, tri[:], val=1.0, diag=True)
    nc.gpsimd.memset(v_sb[:, :, :, HD:HD + 1], 1.0)

    # Score banks and p tiles coexist with the QKV-phase pools (PSUM split
    # 4+4) so QK+exp for early query slices overlap the tail of the QKV
    # phase; only the PV/proj accumulators wait for the QKV banks to free.
    s_pool = ctx.enter_context(
        tc.tile_pool(name=f"s_ps{rep}", bufs=2, space="PSUM"))
    ppool = ctx.enter_context(tc.tile_pool(name=f"psb{rep}", bufs=50))

    # ---------------- Phase 1: QKV projections ----------------
    with tc.tile_pool(name=f"wqkv{rep}", bufs=1) as wpool, \
         tc.tile_pool(name=f"xs{rep}", bufs=2) as xpool, \
         tc.tile_pool(name=f"ps1{rep}", bufs=4, space="PSUM") as ps1:
        w_tiles = {}
        for wname, wap in (("wq", wq), ("wk", wk), ("wv", wv)):
            for dt in range(8):
                t = wpool.tile([128, M], BF16, tag=f"{wname}{dt}")
                nc.gpsimd.dma_start(t[:], wap[128 * dt:128 * dt + 128, :])
                w_tiles[(wname, dt)] = t
        p_refs = {}
        first_uses = [2]  # pre-zero the two score buffers' first uses

        def emit_score_group(ti, hp, g, c2):
            h = 2 * hp + c2
            lo = 64 * c2
            s_ps = s_pool.tile([128, 1024], F32, tag="s",
                               name=f"sps_{rep}_{ti}_{hp}_{g}_{c2}")
            if first_uses[0] > 0:
                # later uses hold bounded old scores; first use could hold
                # junk that exp would turn into Inf in the (unread) p region
                first_uses[0] -= 1
                nc.vector.memset(s_ps[:], 0.0)
            for c in range(2):
                jt = 2 * g + c
                # rows below a diagonal tile's start are fully masked and
                # never read downstream: skip them (bf16 is 1 cyc/row at
                # any width, unlike fp32r)
                off = 128 * (jt - 4 * ti) if jt >= 4 * ti else 0
                nc.tensor.matmul(
                    s_ps[:, 512 * c + off:512 * c + 512],
                    k_sb[lo:lo + 64, hp, 128 * jt:128 * jt + 128],
                    q_sb[lo:lo + 64, hp, 512 * ti + off:512 * ti + 512],
                    start=True, stop=True)
            p_t = ppool.tile([128, 1024], BF16, tag="p",
                             name=f"pt_{rep}_{ti}_{hp}_{g}_{c2}")
            off0 = 128 * (2 * g - 4 * ti) if 2 * g >= 4 * ti else 0
            nc.scalar.activation(p_t[:, off0:1024], s_ps[:, off0:1024],
                                 EXP, scale=0.125)
            for c in range(2):
                jt = 2 * g + c
                if jt >= 4 * ti:
                    off = 128 * (jt - 4 * ti)
                    band = p_t[:, 512 * c + off:512 * c + off + 128]
                    nc.vector.tensor_mul(band, band, tri[:])
            p_refs[(ti, hp, g, c2)] = p_t
            return p_t

        xt_r = xt.rearrange("(dt p) s -> p dt s", p=128)
        for ss in range(4):
            xt_t = xpool.tile([128, 8, 512], BF16, tag="x")
            for dh in range(4):
                nc.sync.dma_start(
                    xt_t[:, 2 * dh:2 * dh + 2, :],
                    xt_r[:, 2 * dh:2 * dh + 2, 512 * ss:512 * ss + 512])
            for wname, dst in (("wq", q_sb), ("wk", k_sb)):
                for mt in range(4):
                    ps = ps1.tile([128, 512], F32, tag="ps")
                    for dt in range(8):
                        nc.tensor.matmul(
                            ps[:], w_tiles[(wname, dt)][:, 128 * mt:128 * mt + 128],
                            xt_t[:, dt, :], start=(dt == 0), stop=(dt == 7))
                    nc.vector.tensor_copy(dst[:, mt, 512 * ss:512 * ss + 512], ps[:])
            for st in range(4):
                ps = ps1.tile([128, 512], F32, tag="ps")
                for dt in range(8):
                    nc.tensor.matmul(
                        ps[:], xt_t[:, dt, 128 * st:128 * st + 128],
                        w_tiles[("wv", dt)][:], start=(dt == 0), stop=(dt == 7))
                jt = 4 * ss + st
                nc.vector.tensor_copy(
                    v_sb[:, jt, :, 0:HD], ps[:].rearrange("p (h d) -> p h d", h=NHC))
            if ss <= 2:
                # early query-slices' scores+exp feed the otherwise-idle
                # ScalarE during the QKV phase
                for hp0 in range(4):
                    for g0 in range(2 * (ss + 1)):
                        for c20 in range(2):
                            emit_score_group(ss, hp0, g0, c20)

    # ---------------- Phase 2: attention + output projection ----------------
    with tc.tile_pool(name=f"wp{rep}", bufs=1) as wppool, \
         tc.tile_pool(name=f"p2sb{rep}", bufs=1) as p2pool, \
         tc.tile_pool(name=f"misc{rep}", bufs=4) as mpool, \
         tc.tile_pool(name=f"osb{rep}", bufs=3) as opool, \
         tc.tile_pool(name=f"dscr{rep}", bufs=8, space="DRAM") as dpool, \
         tc.tile_pool(name=f"y_ps{rep}", bufs=3, space="PSUM") as y_pool, \
         tc.tile_pool(name=f"o_ps{rep}", bufs=1, space="PSUM") as o_pool:

        # attention output, transposed: [dh-of-2-heads (128), dh-tile, seq]
        y_sb = p2pool.tile([128, 4, SEQ], BF16, tag="y")
        wp_sb = []
        for dt in range(4):
            wt = wppool.tile([128, DM], BF16, tag=f"wp{dt}")
            nc.sync.dma_start(wt[:], wp[128 * dt:128 * dt + 128, :])
            wp_sb.append(wt)

        for ti in range(NTI):
            n_j = 4 * (ti + 1)
            ng = n_j // 2
            for hp in range(4):
                y_tiles = [y_pool.tile([128, 512], F32, tag="y", name=f"yps_{rep}_{ti}_{hp}_{i}")
                           for i in range(2)]
                for g in range(ng):
                    for c2 in range(2):
                        p_t = p_refs.get((ti, hp, g, c2))
                        if p_t is None:
                            p_t = emit_score_group(ti, hp, g, c2)
                        h = 2 * hp + c2
                        for c in range(2):
                            jt = 2 * g + c
                            # columns below a diagonal tile's start are fully
                            # masked: skip them instead of zeroing p there
                            off = 128 * (jt - 4 * ti) if jt >= 4 * ti else 0
                            nc.tensor.matmul(
                                y_tiles[c2][0:HD + 1, off:512],
                                v_sb[:, jt, h, :],
                                p_t[:, 512 * c + off:512 * c + 512],
                                start=(g == 0 and c == 0), stop=(jt == n_j - 1),
                                skip_group_check=True)
                for c2 in range(2):
                    lo = 64 * c2
                    y_ps = y_tiles[c2]
                    rc = mpool.tile([1, 512], F32, tag="rc")
                    nc.vector.reciprocal(rc[:], y_ps[HD:HD + 1, :])
                    scr = dpool.tile([1, 512], F32, tag="scr")
                    nc.sync.dma_start(scr[:], rc[:])
                    bc = mpool.tile([64, 512], F32, tag="bc")
                    nc.sync.dma_start(bc[:], scr[:].to_broadcast([64, 512]))
                    nc.vector.tensor_mul(
                        y_sb[lo:lo + 64, hp, 512 * ti:512 * ti + 512],
                        y_ps[0:HD, :], bc[:])
            for st in range(4):
                s0 = 512 * ti + 128 * st
                for e in range(2):
                    op = o_pool.tile([128, 512], F32, tag="o")
                    for dt in range(4):
                        nc.tensor.matmul(
                            op[:], y_sb[:, dt, s0:s0 + 128],
                            wp_sb[dt][:, 512 * e:512 * e + 512],
                            start=(dt == 0), stop=(dt == 3))
                    ot = opool.tile([128, 512], BF16, tag="ot")
                    nc.vector.tensor_copy(ot[:], op[:])
                    nc.gpsimd.dma_start(out[s0:s0 + 128, 512 * e:512 * e + 512], ot[:])


_NC = None


def _get_nc():
    global _NC
    if _NC is None:
        _NC = _build_nc()
    return _NC


def _core_inputs(x, w_qkv, w_proj, core):
    b, g = core // 2, core % 2
    ms = slice(512 * g, 512 * g + 512)
    return {
        "xt": np.ascontiguousarray(x[b].T.astype(ml_dtypes.bfloat16)),
        "wq": np.ascontiguousarray(w_qkv[0:1024][ms].T.astype(ml_dtypes.bfloat16)),
        "wk": np.ascontiguousarray(w_qkv[1024:2048][ms].T.astype(ml_dtypes.bfloat16)),
        "wv": np.ascontiguousarray(w_qkv[2048:3072][ms].T.astype(ml_dtypes.bfloat16)),
        "wp": np.ascontiguousarray(w_proj[:, ms].T.astype(ml_dtypes.bfloat16)),
    }


def kernel(x, w_qkv, w_proj):
    x = np.asarray(x, dtype=np.float32)
    w_qkv = np.asarray(w_qkv, dtype=np.float32)
    w_proj = np.asarray(w_proj, dtype=np.float32)
    nc = _get_nc()
    in_maps = [_core_inputs(x, w_qkv, w_proj, c) for c in range(8)]
    res = run_bass_kernel_spmd(nc, in_maps, core_ids=list(range(8)))
    out = np.empty((4, SEQ, DM), dtype=np.float32)
    for b in range(4):
        out[b] = (res.results[2 * b]["out"].astype(np.float32)
                  + res.results[2 * b + 1]["out"].astype(np.float32))
    return out


if __name__ == "__main__":
    rng = np.random.default_rng(0)
    x = rng.standard_normal((4, SEQ, DM), dtype=np.float32)
    w_qkv = (rng.random((3 * DM, DM), dtype=np.float32) - 0.5) / 16.0
    w_proj = (rng.random((DM, DM), dtype=np.float32) - 0.5) / 16.0
    y = kernel(x, w_qkv, w_proj)
    print("ok", y.shape, float(np.abs(y).mean()))
